# revision 14
# baseline (speedup 1.0000x reference)
"""Trainium2 Bass kernel for nn_Block_82042465288934 (involution block).

Per-core layout: data-parallel over batch (B=8 images over 8 cores), one
image per core, channel-major [c=128 partitions, h*w=4096 free].

Pipeline per core:
  conv1+BN+ReLU folded on host -> PE matmul + ACT Relu -> t2 [32,4096] bf16
  involution weights: per-tap matmul with host-replicated conv2 rows
    (lhsT rows repeated x16 across group channels) -> PSUM holds w_rep
  ACT evacuates PSUM -> SBUF bf16 (fused +conv2 bias)
  DVE: 49-tap multiply-accumulate in bf16 (2x mode; two x_pad copies at
    byte offsets 0/+1 keep every tap 4B-aligned)
  LN: PE ones-matmul channel sums -> DMA reshape [1,8192]->[128,64] ->
    stats math -> DMA back -> K=1 broadcast matmuls -> DVE normalize
  MLP: PE matmuls (LN affine folded into pw1 on host), ACT erf-Gelu,
    residual + pw2 bias via one fused scalar_tensor_tensor.
"""

import numpy as np
import ml_dtypes

B, DIM, H, W = 8, 128, 64, 64
K = 7
PAD = 3
GC = 16
G = 8
RED = 4
HID = DIM // RED          # 32
N = H * W                 # 4096
NT = K * K                # 49 taps
HP = H + 2 * PAD          # 70 (padded row stride)
BN_EPS = 1e-5
LN_EPS = 1e-6
F2 = 2 * DIM              # 256

_BUILD_CACHE = {}

bf16 = ml_dtypes.bfloat16


def _build():
    """Trace + compile the single-core bass kernel. Cached per process."""
    if "nc" in _BUILD_CACHE:
        return _BUILD_CACHE["nc"]

    import concourse.bacc as bacc
    import concourse.tile as tile
    from concourse import mybir

    f32 = mybir.dt.float32
    b16 = mybir.dt.bfloat16
    AF = mybir.ActivationFunctionType
    OP = mybir.AluOpType

    nc = bacc.Bacc("TRN2", target_bir_lowering=False, debug=False, num_devices=1)

    # ---- DRAM I/O ----
    x_d = nc.dram_tensor("x", (DIM, N), f32, kind="ExternalInput")
    w1T_d = nc.dram_tensor("w1T", (DIM, HID), f32, kind="ExternalInput")
    b1_d = nc.dram_tensor("b1", (HID, 1), f32, kind="ExternalInput")
    c2wT_d = nc.dram_tensor("c2wT", (HID, NT * DIM), b16, kind="ExternalInput")
    c2b_d = nc.dram_tensor("c2b", (DIM, NT), f32, kind="ExternalInput")
    onesc_d = nc.dram_tensor("onesc", (DIM, 1), b16, kind="ExternalInput")
    onesr_d = nc.dram_tensor("onesr", (1, DIM), f32, kind="ExternalInput")
    w1pT_d = nc.dram_tensor("w1pT", (DIM, F2), b16, kind="ExternalInput")
    b1p_d = nc.dram_tensor("b1p", (DIM, 2), f32, kind="ExternalInput")
    w2T_d = nc.dram_tensor("w2T", (DIM, F2), b16, kind="ExternalInput")
    b2_d = nc.dram_tensor("b2", (DIM, 1), f32, kind="ExternalInput")
    out_d = nc.dram_tensor("out", (DIM, N), f32, kind="ExternalOutput")

    with tile.TileContext(nc) as tc:
        with (
            tc.tile_pool(name="const", bufs=1) as const,
            tc.tile_pool(name="work", bufs=3) as work,
            tc.tile_pool(name="psum", bufs=2, space="PSUM") as psum,
        ):
            # ---- load inputs ----
            x_sb = const.tile([DIM, N], f32)
            nc.sync.dma_start(out=x_sb[:], in_=x_d.ap())
            w1T_sb = const.tile([DIM, HID], f32)
            nc.sync.dma_start(out=w1T_sb[:], in_=w1T_d.ap())
            b1_sb = const.tile([HID, 1], f32)
            nc.sync.dma_start(out=b1_sb[:], in_=b1_d.ap())
            c2wT_sb = const.tile([HID, NT * DIM], b16)
            nc.sync.dma_start(out=c2wT_sb[:], in_=c2wT_d.ap())
            c2b_sb = const.tile([DIM, NT], f32)
            nc.sync.dma_start(out=c2b_sb[:], in_=c2b_d.ap())
            onesc_sb = const.tile([DIM, 1], b16)
            nc.sync.dma_start(out=onesc_sb[:], in_=onesc_d.ap())
            onesr_sb = const.tile([1, DIM], f32)
            nc.sync.dma_start(out=onesr_sb[:], in_=onesr_d.ap())
            w1pT_sb = const.tile([DIM, F2], b16)
            nc.sync.dma_start(out=w1pT_sb[:], in_=w1pT_d.ap())
            b1p_sb = const.tile([DIM, 2], f32)
            nc.sync.dma_start(out=b1p_sb[:], in_=b1p_d.ap())
            w2T_sb = const.tile([DIM, F2], b16)
            nc.sync.dma_start(out=w2T_sb[:], in_=w2T_d.ap())
            b2_sb = const.tile([DIM, 1], f32)
            nc.sync.dma_start(out=b2_sb[:], in_=b2_d.ap())

            # ---- padded bf16 copies of x (offset 0 and +1 element for
            #      4B alignment of every tap) ----
            xp0 = const.tile([DIM, HP * HP], b16)
            xp1 = const.tile([DIM, HP * HP + 2], b16)
            nc.gpsimd.memset(xp0[:], 0.0)
            nc.gpsimd.memset(xp1[:], 0.0)
            xp0v = xp0[:].rearrange("p (a b) -> p a b", a=HP, b=HP)
            xp1v = xp1[:, 1 : 1 + HP * HP].rearrange("p (a b) -> p a b", a=HP, b=HP)
            xv = x_sb[:].rearrange("p (a b) -> p a b", a=H, b=W)
            nc.vector.tensor_copy(out=xp0v[:, PAD : PAD + H, PAD : PAD + W], in_=xv)
            nc.vector.tensor_copy(out=xp1v[:, PAD : PAD + H, PAD : PAD + W], in_=xv)

            # ---- conv1 + BN + ReLU -> t2 [HID, N] bf16 ----
            t2_sb = const.tile([HID, N], b16)
            for s in range(N // 512):
                pc1 = psum.tile([HID, 512], f32, tag="ps")
                nc.tensor.matmul(
                    out=pc1[:],
                    lhsT=w1T_sb[:],
                    rhs=x_sb[:, s * 512 : (s + 1) * 512],
                )
                nc.scalar.activation(
                    out=t2_sb[:, s * 512 : (s + 1) * 512],
                    in_=pc1[:],
                    func=AF.Relu,
                    bias=b1_sb[:],
                )

            # ---- involution: 49-tap accumulate ----
            acc_sb = const.tile([DIM, N], b16)
            accv = acc_sb[:].rearrange("p (a b) -> p a b", a=H, b=W)
            HHALF = H // 2  # 32 rows per half-chunk (2048 elems)
            for t in range(NT):
                di, dj = t // K, t % K
                # flat element offset of the window is (di+h)*70 + dj + w;
                # parity = dj parity (70 is even). Odd dj reads the +1-shifted
                # copy so the innermost run stays 4-byte aligned.
                xsrc_v = xp0v if dj % 2 == 0 else xp1v
                lhsT_t = c2wT_sb[:, t * DIM : (t + 1) * DIM]
                for half in range(2):
                    pw = psum.tile([DIM, 2048], f32, tag="ps")
                    for s in range(4):
                        n0 = half * 2048 + s * 512
                        nc.tensor.matmul(
                            out=pw[:, s * 512 : (s + 1) * 512],
                            lhsT=lhsT_t,
                            rhs=t2_sb[:, n0 : n0 + 512],
                        )
                    wrep = work.tile([DIM, 2048], b16, tag="wrep")
                    nc.scalar.activation(
                        out=wrep[:],
                        in_=pw[:],
                        func=AF.Identity,
                        bias=c2b_sb[:, t : t + 1],
                    )
                    wrepv = wrep[:].rearrange("p (a b) -> p a b", a=HHALF, b=W)
                    # shifted x window for this tap / half (rows
                    # half*32..half*32+31 of the output)
                    r0 = di + half * HHALF
                    xs = xsrc_v[:, r0 : r0 + HHALF, dj : dj + W]
                    av = accv[:, half * HHALF : (half + 1) * HHALF, :]
                    if t == 0:
                        nc.vector.tensor_mul(av, wrepv, xs)
                    else:
                        prod = work.tile([DIM, HHALF, W], b16, tag="prod")
                        nc.vector.tensor_mul(prod[:], wrepv, xs)
                        nc.vector.tensor_add(av, av, prod[:])

            # ---- LayerNorm stats: channel sums via ones-matmul ----
            zero_t = const.tile([DIM, 1], f32)
            nc.vector.memset(zero_t[:], 0.0)
            y2_sb = const.tile([DIM, N], b16)
            nc.scalar.activation(
                out=y2_sb[:], in_=acc_sb[:], func=AF.Square, bias=zero_t[:]
            )
            stats_row = const.tile([1, 2 * N], f32)
            for half in range(2):
                ps1 = psum.tile([1, 2048], f32, tag="ps")
                ps2 = psum.tile([1, 2048], f32, tag="ps")
                for s in range(4):
                    n0 = half * 2048 + s * 512
                    nc.tensor.matmul(
                        out=ps1[:, s * 512 : (s + 1) * 512],
                        lhsT=onesc_sb[:],
                        rhs=acc_sb[:, n0 : n0 + 512],
                    )
                    nc.tensor.matmul(
                        out=ps2[:, s * 512 : (s + 1) * 512],
                        lhsT=onesc_sb[:],
                        rhs=y2_sb[:, n0 : n0 + 512],
                    )
                nc.scalar.copy(
                    out=stats_row[:, half * 2048 : (half + 1) * 2048], in_=ps1[:]
                )
                nc.scalar.copy(
                    out=stats_row[:, N + half * 2048 : N + (half + 1) * 2048],
                    in_=ps2[:],
                )

            # reshape [1, 2*4096] -> [128, 2, 32] (s1 | s2 per pixel strip)
            SJ = N // DIM  # 32
            stats_t = const.tile([DIM, 2, SJ], f32)
            for k in range(2):
                nc.sync.dma_start(
                    out=stats_t[:, k, :],
                    in_=stats_row[:, k * N : (k + 1) * N].rearrange(
                        "o (p j) -> o p j", p=DIM, j=SJ
                    ),
                )
            # stats math on [128, 32]
            mr_t = const.tile([DIM, 2, SJ], f32)  # [:,0,:]=mu  [:,1,:]=rstd
            mu_t = mr_t[:, 0, :]
            nc.vector.tensor_scalar(
                out=mu_t, in0=stats_t[:, 0, :], scalar1=1.0 / DIM, scalar2=None,
                op0=OP.mult,
            )
            m2_t = const.tile([DIM, SJ], f32)
            nc.vector.tensor_mul(m2_t[:], mu_t, mu_t)
            var_t = const.tile([DIM, SJ], f32)
            nc.vector.scalar_tensor_tensor(
                out=var_t[:], in0=stats_t[:, 1, :], scalar=1.0 / DIM, in1=m2_t[:],
                op0=OP.mult, op1=OP.subtract,
            )
            eps_t = const.tile([DIM, 1], f32)
            nc.vector.memset(eps_t[:], LN_EPS)
            std_t = const.tile([DIM, SJ], f32)
            nc.scalar.activation(
                out=std_t[:], in_=var_t[:], func=AF.Sqrt, bias=eps_t[:]
            )
            nc.vector.reciprocal(out=mr_t[:, 1, :], in_=std_t[:])
            # back to rows [1, 2*4096]: (k p j)
            mr_row = const.tile([1, 2 * N], f32)
            for k in range(2):
                nc.sync.dma_start(
                    out=mr_row[:, k * N : (k + 1) * N].rearrange(
                        "o (p j) -> o p j", p=DIM, j=SJ
                    ),
                    in_=mr_t[:, k, :],
                )

            # ---- normalize: y_norm = (y - mu) * rstd  (broadcast via K=1 matmul)
            yn_sb = const.tile([DIM, N], b16)
            for half in range(2):
                pmu = psum.tile([DIM, 2048], f32, tag="ps")
                for s in range(4):
                    n0 = half * 2048 + s * 512
                    nc.tensor.matmul(
                        out=pmu[:, s * 512 : (s + 1) * 512],
                        lhsT=onesr_sb[:],
                        rhs=mr_row[:, n0 : n0 + 512],
                    )
                yc = work.tile([DIM, 2048], b16, tag="yc")
                nc.vector.tensor_sub(
                    yc[:], acc_sb[:, half * 2048 : (half + 1) * 2048], pmu[:]
                )
                prs = psum.tile([DIM, 2048], f32, tag="ps")
                for s in range(4):
                    n0 = half * 2048 + s * 512
                    nc.tensor.matmul(
                        out=prs[:, s * 512 : (s + 1) * 512],
                        lhsT=onesr_sb[:],
                        rhs=mr_row[:, N + n0 : N + n0 + 512],
                    )
                nc.vector.tensor_mul(
                    yn_sb[:, half * 2048 : (half + 1) * 2048], yc[:], prs[:]
                )

            # ---- MLP + residual ----
            out_sb = const.tile([DIM, N], f32)
            for s in range(N // 512):
                n0 = s * 512
                ph_a = psum.tile([DIM, 512], f32, tag="ps")
                nc.tensor.matmul(
                    out=ph_a[:], lhsT=w1pT_sb[:, 0:DIM], rhs=yn_sb[:, n0 : n0 + 512]
                )
                ha = work.tile([DIM, 512], b16, tag="ha")
                nc.scalar.activation(
                    out=ha[:], in_=ph_a[:], func=AF.Gelu, bias=b1p_sb[:, 0:1]
                )
                ph_b = psum.tile([DIM, 512], f32, tag="ps")
                nc.tensor.matmul(
                    out=ph_b[:], lhsT=w1pT_sb[:, DIM:F2], rhs=yn_sb[:, n0 : n0 + 512]
                )
                hb = work.tile([DIM, 512], b16, tag="hb")
                nc.scalar.activation(
                    out=hb[:], in_=ph_b[:], func=AF.Gelu, bias=b1p_sb[:, 1:2]
                )
                po = psum.tile([DIM, 512], f32, tag="ps")
                nc.tensor.matmul(
                    out=po[:], lhsT=w2T_sb[:, 0:DIM], rhs=ha[:], start=True, stop=False
                )
                nc.tensor.matmul(
                    out=po[:], lhsT=w2T_sb[:, DIM:F2], rhs=hb[:], start=False, stop=True
                )
                nc.vector.scalar_tensor_tensor(
                    out=out_sb[:, n0 : n0 + 512],
                    in0=po[:],
                    scalar=b2_sb[:],
                    in1=x_sb[:, n0 : n0 + 512],
                    op0=OP.add,
                    op1=OP.add,
                )

            nc.sync.dma_start(out=out_d.ap(), in_=out_sb[:])

    nc.compile()
    _BUILD_CACHE["nc"] = nc
    return nc


def _prep_weights(inputs):
    """Host-side folding/packing of all weight tensors (shared by all cores)."""
    f = lambda k: np.asarray(inputs[k], dtype=np.float32)
    conv1_w, conv1_b = f("conv1_w"), f("conv1_b")
    bn_g, bn_b = f("bn_g"), f("bn_b")
    bn_mean, bn_var = f("bn_mean"), f("bn_var")
    conv2_w, conv2_b = f("conv2_w"), f("conv2_b")
    ln_g, ln_b = f("ln_g"), f("ln_b")
    pw1_w, pw1_b = f("pw1_w"), f("pw1_b")
    pw2_w, pw2_b = f("pw2_w"), f("pw2_b")

    s = bn_g / np.sqrt(bn_var + BN_EPS)
    w1f = conv1_w * s[:, None]
    b1f = conv1_b * s + (bn_b - bn_mean * s)

    gidx = np.arange(DIM) // GC  # group of each channel
    # c2wT[r, t*128 + c] = conv2_w[g(c)*49 + t, r]
    c2wT = np.empty((HID, NT * DIM), dtype=np.float32)
    for t in range(NT):
        c2wT[:, t * DIM : (t + 1) * DIM] = conv2_w[gidx * NT + t].T
    c2b_rep = conv2_b[gidx[:, None] * NT + np.arange(NT)[None, :]]  # [128, 49]

    W1p = pw1_w * ln_g[None, :]
    b1p = pw1_b + pw1_w @ ln_b
    b1p2 = np.stack([b1p[:DIM], b1p[DIM:]], axis=1)  # [128, 2]
    # w2T[p, k*128 + c] = pw2_w[c, k*128 + p]
    w2T = np.empty((DIM, F2), dtype=np.float32)
    w2T[:, 0:DIM] = pw2_w.T[0:DIM]
    w2T[:, DIM:F2] = pw2_w.T[DIM:F2]

    return {
        "w1T": np.ascontiguousarray(w1f.T, dtype=np.float32),
        "b1": b1f.reshape(HID, 1).astype(np.float32),
        "c2wT": c2wT.astype(bf16),
        "c2b": np.ascontiguousarray(c2b_rep, dtype=np.float32),
        "onesc": np.ones((DIM, 1), dtype=bf16),
        "onesr": np.ones((1, DIM), dtype=np.float32),
        "w1pT": np.ascontiguousarray(W1p.T, dtype=bf16),
        "b1p": np.ascontiguousarray(b1p2, dtype=np.float32),
        "w2T": w2T.astype(bf16),
        "b2": pw2_b.reshape(DIM, 1).astype(np.float32),
    }


def _get_runner(nc, n_cores):
    """Build (once) the jitted SPMD executable + metadata for running the
    bass module on `n_cores` devices with pre-sharded inputs (avoids XLA
    data-movement modules that the generic neuronx-cc path here cannot
    compile)."""
    if "runner" in _BUILD_CACHE:
        return _BUILD_CACHE["runner"]

    import jax
    from jax.sharding import Mesh, NamedSharding, PartitionSpec
    from jax.experimental.shard_map import shard_map
    from concourse import bass2jax, mybir

    bass2jax.install_neuronx_cc_hook()

    in_names, out_names, out_avals, zero_outs = [], [], [], []
    for alloc in nc.m.functions[0].allocations:
        if not isinstance(alloc, mybir.MemoryLocationSet):
            continue
        name = alloc.memorylocations[0].name
        if alloc.kind == "ExternalInput":
            in_names.append(name)
        elif alloc.kind == "ExternalOutput":
            shape = tuple(alloc.tensor_shape)
            dtype = mybir.dt.np(alloc.dtype)
            out_names.append(name)
            out_avals.append(jax.core.ShapedArray(shape, dtype))
            zero_outs.append(np.zeros(shape, dtype))
    n_params = len(in_names)
    n_outs = len(out_avals)
    all_names = in_names + out_names
    donate = tuple(range(n_params, n_params + n_outs))

    def _body(*args):
        outs = bass2jax._bass_exec_p.bind(
            *args,
            out_avals=tuple(out_avals),
            in_names=tuple(all_names),
            out_names=tuple(out_names),
            lowering_input_output_aliases=(),
            sim_require_finite=True,
            sim_require_nnan=True,
            nc=nc,
        )
        return tuple(outs)

    devices = jax.devices()[:n_cores]
    mesh = Mesh(np.asarray(devices), ("core",))
    in_specs = (PartitionSpec("core"),) * (n_params + n_outs)
    out_specs = (PartitionSpec("core"),) * n_outs
    sharded = jax.jit(
        shard_map(
            _body, mesh=mesh, in_specs=in_specs, out_specs=out_specs, check_rep=False
        ),
        donate_argnums=donate,
        keep_unused=True,
    )

    def make_global(per_core_arrays):
        shards = [
            jax.device_put(np.ascontiguousarray(a), d)
            for a, d in zip(per_core_arrays, devices)
        ]
        shape = (n_cores * shards[0].shape[0],) + tuple(shards[0].shape[1:])
        sharding = NamedSharding(mesh, PartitionSpec("core"))
        return jax.make_array_from_single_device_arrays(shape, sharding, shards)

    # partition_id is auto-declared by bass; feed each core its index.
    pid_name = nc.partition_id_tensor.name if nc.partition_id_tensor else None
    pid_shape, pid_dtype = None, None
    if pid_name is not None:
        for alloc in nc.m.functions[0].allocations:
            if (
                isinstance(alloc, mybir.MemoryLocationSet)
                and alloc.memorylocations[0].name == pid_name
            ):
                pid_shape = tuple(alloc.tensor_shape)
                pid_dtype = mybir.dt.np(alloc.dtype)

    runner = {
        "sharded": sharded,
        "make_global": make_global,
        "in_names": in_names,
        "out_names": out_names,
        "out_avals": out_avals,
        "zero_outs": zero_outs,
        "n_cores": n_cores,
        "pid": (pid_name, pid_shape, pid_dtype),
    }
    _BUILD_CACHE["runner"] = runner
    return runner


def _run_spmd(nc, in_maps):
    r = _get_runner(nc, len(in_maps))
    n_cores = r["n_cores"]
    pid_name, pid_shape, pid_dtype = r["pid"]
    if pid_name is not None:
        for c, m in enumerate(in_maps):
            m[pid_name] = np.full(pid_shape, c, dtype=pid_dtype)
    make_global = r["make_global"]
    args = [make_global([m[name] for m in in_maps]) for name in r["in_names"]]
    args += [make_global([z] * n_cores) for z in r["zero_outs"]]
    out_arrs = r["sharded"](*args)
    results = []
    for c in range(n_cores):
        results.append(
            {
                name: np.asarray(out_arrs[i].addressable_shards[c].data)
                for i, name in enumerate(r["out_names"])
            }
        )
    return results


def kernel(**inputs) -> np.ndarray:
    nc = _build()
    weights = _prep_weights(inputs)
    x = np.asarray(inputs["x"], dtype=np.float32).reshape(B, DIM, N)

    in_maps = []
    for b in range(B):
        m = dict(weights)
        m["x"] = np.ascontiguousarray(x[b])
        in_maps.append(m)

    results = _run_spmd(nc, in_maps)
    out = np.stack([r["out"] for r in results]).reshape(B, DIM, H, W)
    return out.astype(np.float32)


if __name__ == "__main__":
    _build()
    print("build ok")
